# revision 72
# baseline (speedup 1.0000x reference)
"""Mistral attention (B=1, S=2048, H=4096, 32 q-heads / 8 kv-heads GQA,
RoPE, causal) on 8 trn2 NeuronCores.

Sharding: tensor-parallel by kv head. Core c owns kv head c, q heads
4c..4c+3, and Wo rows 512c..512c+512. Attention outputs are AllGathered
per 512-token chunk; each core then computes its 512-row slice of the
output projection.

Schedule: chunk-serial and fully pipelined: for each 512-token chunk,
Q-projection -> (RoPE-q hidden under the K/V projection) -> attention
-> AllGather, so the four AllGathers cascade behind later chunks'
compute; per-chunk output-projection passes run at the end, each gated
only by its (long-finished) AllGather. The front is HBM-bound (~0.4
GB/us): chunk 0 interleaves kv 8-ht matmul groups AND chunk 1's q
heads 0/1 (same supertiles + weights, zero extra DMA, accumulated in
the otherwise-idle pc/pd banks and staged to SBUF) into the Q pass,
and the input DMA cascade is issued in exact PE-consumption order at
~0.5-1MB granularity over both hwdge queues, so the PE starts ~11us in
and streams behind the fill. proj1 then runs h2/h3 only, which also
frees it from the p01-bank WAR on attn0's last softmax exps. The AllGather payload is rank-major so
each o-proj read is one 4KB-row DMA per rank.

Attention is kt-outer/head-inner (ascending kt: diagonal tiles last,
hiding RoPE-k), with AV matmuls lagging scores by one kt step.
Masked diagonal tiles are trimmed to live query columns (N=512-128m).
Softmax denominators accumulate on the vector engine; the per-head
tail is entirely off the PE: GPSIMD partition_all_reduce -> DVE
reciprocal -> DVE multiply reading the AV psum bank directly, half
emitted inline and half woven into the next chunk's Q pass. Softmax
skips max-subtraction (unit-scale inputs). O-proj double-buffers its
accumulators across chunks (p01/p23 vs pa-pd), runs the last rank
o-major so read-out overlaps accumulation, and spreads store DMAs over
sync/scalar/gpsimd. Value path bf16; PSUM accumulation fp32; output
written bf16 and upcast on host.
"""

import math

import ml_dtypes
import numpy as np

P = 128
S = 2048
H = 4096
HD = 128
NQH = 4  # q heads per core
TC = 512  # token chunk
NT = S // TC  # 4 chunks
HT = H // P  # 32 h tiles
N_CORES = 8
ROPE_THETA = 10000.0

_BUILT = None


def _rope_tables():
    """cosT/sin2T in [hd partition, token free] layout.

    sin2T is the sin table pre-shifted/signed so that
    q_rot = q*cosT + shift128(q*sin2T), where shift128 swaps the two
    64-partition halves.
    """
    inv_freq = 1.0 / (ROPE_THETA ** (np.arange(0, HD, 2, dtype=np.float64) / HD))
    t = np.arange(S, dtype=np.float64)
    freqs = np.outer(t, inv_freq)  # [S, 64]
    emb = np.concatenate([freqs, freqs], axis=1)  # [S, HD]
    cosT = np.cos(emb).T.astype(np.float32)  # [HD, S]
    sinT = np.sin(emb).T.astype(np.float32)
    sin2T = sinT.copy()
    sin2T[64:] = -sin2T[64:]
    return (
        np.ascontiguousarray(cosT).astype(ml_dtypes.bfloat16),
        np.ascontiguousarray(sin2T).astype(ml_dtypes.bfloat16),
    )


def _mask():
    """[128, 512] bf16: mask[i, j] = (j >= i). Diagonal tile m of a chunk
    uses mask[:, 0:512-128m] against query columns [128m, 512)."""
    i = np.arange(P)[:, None]
    j = np.arange(TC)[None, :]
    return np.ascontiguousarray((j >= i).astype(np.float32)).astype(ml_dtypes.bfloat16)


def _build():
    import concourse.bacc as bacc
    import concourse.mybir as mybir
    import concourse.tile as tile

    f32 = mybir.dt.float32
    f32r = mybir.dt.float32r
    bf16 = mybir.dt.bfloat16

    nc = bacc.Bacc(
        "TRN2", target_bir_lowering=False, debug=False, num_devices=N_CORES
    )

    # Host-side repacked layouts: partition-major [128, ...] with wide
    # contiguous rows so DMA descriptors are 2KB+ (1KB rows cap a DMA
    # queue at ~90GB/s; the kernel front is load-bound otherwise).
    hs2 = nc.declare_dram_parameter("hs2", [P, HT, S], bf16, isOutput=False)
    wq2 = nc.declare_dram_parameter("wq2", [P, HT * NQH * HD], bf16, isOutput=False)
    wk2 = nc.declare_dram_parameter("wk2", [P, HT * HD], bf16, isOutput=False)
    wv2 = nc.declare_dram_parameter("wv2", [P, HT * HD], bf16, isOutput=False)
    wo2 = nc.declare_dram_parameter("wo2", [P, HT * NQH * HD], bf16, isOutput=False)
    out_ext = nc.declare_dram_parameter("out", [NQH * HD, S], bf16, isOutput=True)

    cosT_np, sin2T_np = _rope_tables()
    cos_dram = nc.inline_tensor(cosT_np, name="cosT")
    sin_dram = nc.inline_tensor(sin2T_np, name="sin2T")
    mask_dram = nc.inline_tensor(_mask(), name="mask")
    id_dram = nc.inline_tensor(np.eye(P).astype(ml_dtypes.bfloat16), name="ident")

    # rank-major AllGather payload: [hd, (head, token)] per rank, so the
    # o-proj phase can read one rank's 4 heads as a single 4KB-row DMA
    ag_in = [nc.dram_tensor(f"ag_in{c}", [P, NQH * TC], bf16) for c in range(NT)]
    ag_out = [
        nc.dram_tensor(f"ag_out{c}", [N_CORES * P, NQH * TC], bf16, addr_space="Shared")
        for c in range(NT)
    ]

    Exp = mybir.ActivationFunctionType.Exp
    SCALE = 1.0 / math.sqrt(HD)

    with tile.TileContext(nc) as tc:
        with (
            tc.tile_pool(name="const", bufs=1) as constp,
            tc.tile_pool(name="qkvout", bufs=1) as qp,
            tc.tile_pool(name="pmain", bufs=1, space="PSUM") as pm,
        ):
            # constants
            cos_sb = constp.tile([P, S], bf16)
            sin_sb = constp.tile([P, S], bf16)
            mask_sb = constp.tile([P, TC], bf16)
            mask2_sb = constp.tile([P, 2 * TC], bf16)
            id_sb = constp.tile([P, P], bf16)

            # persistent qkv outputs (bf16: PE runs bf16 at full rate)
            qT_sb = qp.tile([P, NQH * S], bf16)  # [hd, (head, t)]
            kT_sb = qp.tile([P, S], bf16)
            vnat_sb = qp.tile([P, S], bf16)  # [t%128, (ttile, hd)]

            # PSUM: 8 banks as two 2-bank tiles (p01, p23) and four 1-bank
            # tiles (pa..pd). Explicit tags keep cross-phase deps per-bank.
            def p2(tag, name):
                return pm.tile([P, 2 * TC], f32, tag=tag, bufs=1, name=name)

            def p1(tag, name):
                return pm.tile([P, TC], f32, tag=tag, bufs=1, name=name)

            with (
                tc.tile_pool(name="wqkv", bufs=1) as wp,
                tc.tile_pool(name="hsp", bufs=4) as hsp,
                tc.tile_pool(name="workA", bufs=2) as workp,
            ):
                # wq as four quarter tiles: each 8-ht group of the Q pass
                # gates on its own 1.05MB DMA, not half the 4.2MB load
                wq_sbs = [
                    wp.tile([P, HT * NQH * HD // 4], bf16, name=f"wq_sb{i}")
                    for i in range(4)
                ]
                wk_sb = wp.tile([P, HT * HD], bf16)
                wv_sb = wp.tile([P, HT * HD], bf16)

                def attn(c):
                    """Attention for chunk c + its AllGather."""
                    nkt = 4 * (c + 1)
                    avt = ["pa", "pb", "pc", "pd"]
                    av = [p1(avt[h], f"av_{c}_{h}") for h in range(NQH)]
                    ds = [
                        workp.tile([P, TC], bf16, tag=f"ds{h}", bufs=1,
                                   name=f"ds_{c}_{h}")
                        for h in range(NQH)
                    ]
                    pend = None  # (exs, coff, ncols, kt) awaiting AV matmuls

                    def emit_av(p):
                        exv, coff, kt = p
                        for h in range(NQH):
                            nc.tensor.matmul(
                                av[h][:, coff:TC],
                                vnat_sb[:, kt * P : (kt + 1) * P],
                                exv[h],
                                start=(kt == 0),
                                stop=(kt == nkt - 1),
                            )

                    for kt in range(nkt):
                        m = kt - 4 * c
                        ncols = TC - 128 * m if m > 0 else TC
                        coff = TC - ncols
                        scp = p2("p01", f"scp_{c}_{kt}")
                        scq = p2("p23", f"scq_{c}_{kt}")
                        halves = [
                            scp[:, 0:TC], scp[:, TC : 2 * TC],
                            scq[:, 0:TC], scq[:, TC : 2 * TC],
                        ]
                        for h in range(NQH):
                            nc.tensor.matmul(
                                halves[h][:, coff:TC],
                                kT_sb[:, kt * P : (kt + 1) * P],
                                qT_sb[:, h * S + c * TC + coff : h * S + (c + 1) * TC],
                                start=True,
                                stop=True,
                            )
                        if pend is not None:
                            emit_av(pend)
                        ex01 = workp.tile([P, 2 * TC], bf16, tag="ex", bufs=4,
                                          name=f"ex01_{c}_{kt}")
                        ex23 = workp.tile([P, 2 * TC], bf16, tag="ex", bufs=4,
                                          name=f"ex23_{c}_{kt}")
                        if coff == 0:
                            nc.scalar.activation(ex01[:], scp[:], Exp, scale=SCALE)
                            nc.scalar.activation(ex23[:], scq[:], Exp, scale=SCALE)
                        else:
                            for ex, sc in ((ex01, scp), (ex23, scq)):
                                nc.scalar.activation(
                                    ex[:, 0:ncols], sc[:, coff:TC], Exp, scale=SCALE
                                )
                                nc.scalar.activation(
                                    ex[:, TC : TC + ncols], sc[:, TC + coff : 2 * TC],
                                    Exp, scale=SCALE,
                                )
                        exv = [ex01[:, 0:ncols], ex01[:, TC : TC + ncols],
                               ex23[:, 0:ncols], ex23[:, TC : TC + ncols]]
                        if m == 0:
                            nc.vector.tensor_mul(ex01[:], ex01[:], mask2_sb[:])
                            nc.vector.tensor_mul(ex23[:], ex23[:], mask2_sb[:])
                        elif m > 0:
                            for h in range(NQH):
                                nc.vector.tensor_mul(
                                    exv[h], exv[h], mask_sb[:, 0:ncols]
                                )
                        for h in range(NQH):
                            if kt == 0:
                                nc.vector.tensor_copy(ds[h][:], exv[h])
                            else:
                                nc.vector.tensor_add(
                                    ds[h][:, coff:TC], ds[h][:, coff:TC], exv[h]
                                )
                        pend = (exv, coff, kt)
                    emit_av(pend)

                    # per-head tail, all off the PE: denominator via GPSIMD
                    # partition_all_reduce (idle engine; ~3us/head, hidden in
                    # the next chunk's Q pass), then one DVE divide reading
                    # the AV psum bank directly. Heads 0-1 emit here; heads
                    # 2-3 + the AllGather return as a closure the caller
                    # weaves into the next phase's instruction stream.
                    import bass_rust

                    def tail_pair(pair):
                        dnbs = {}
                        for h in pair:
                            dnb = workp.tile([P, TC], f32, tag=f"dnb{h}",
                                             bufs=1, name=f"dnb_{c}_{h}")
                            nc.gpsimd.partition_all_reduce(
                                dnb[:], ds[h][:], channels=P,
                                reduce_op=bass_rust.ReduceOp.add,
                            )
                            dnbs[h] = dnb
                        for h in pair:
                            rcf = workp.tile([P, TC], f32, tag=f"rcf{h % 2}",
                                             bufs=2, name=f"rcf_{c}_{h}")
                            nc.vector.reciprocal_approx_fast(rcf[:], dnbs[h][:])
                            ao = workp.tile([P, TC], bf16, tag="ao", bufs=4,
                                            name=f"ao_{c}_{h}")
                            nc.vector.tensor_mul(ao[:], av[h][:], rcf[:])
                            nc.sync.dma_start(
                                out=ag_in[c][:, h * TC : (h + 1) * TC], in_=ao[:]
                            )

                    tail_pair((0, 1))

                    def finish():
                        tail_pair((2, 3))
                        nc.gpsimd.collective_compute(
                            "AllGather",
                            mybir.AluOpType.bypass,
                            ins=[ag_in[c][:]],
                            outs=[ag_out[c][:]],
                            replica_groups=[list(range(N_CORES))],
                        )

                    return finish

                def emit_loads(pi, ca, cb):
                    """hs for the pair: four [128, 8x1024] supertiles (2KB
                    rows; each dma_start sprays 16 SDMA engines).

                    First pair: the front is HBM-bandwidth-bound (~0.36
                    GB/us), so pieces are issued in exact PE-consumption
                    order of the Q pass (wq quarter + its two hs half-
                    supertiles, per 8-ht group), round-robined over the three
                    DMA-capable queues (sync/scalar/gpsimd) so the aggregate
                    landing order tracks issue order. wk/wv/constants follow
                    (KV pass starts ~35us in, RoPE/masks later still)."""
                    hs_t = {}
                    tiles = {}
                    for g in range(0, HT, 8):
                        tiles[g] = hsp.tile([P, 8 * 2 * TC], bf16, tag="hs",
                                            name=f"hs_{ca}_{g}")
                    if pi == 0:
                        qw = HT * NQH * HD // 4

                        def hsp_half(g, h):
                            return (
                                tiles[g][:, h * 4 * 1024 : (h + 1) * 4 * 1024],
                                hs2[:, g + 4 * h : g + 4 * h + 4,
                                    ca * TC : (cb + 1) * TC],
                            )

                        def wqp(i):
                            return (wq_sbs[i][:], wq2[:, i * qw : (i + 1) * qw])

                        # Two-queue cascade (~0.2 GB/us each): pieces strictly
                        # in PE-consumption order of proj_chunk0's interleaved
                        # [q grp | kv grp] schedule, alternating queues so the
                        # aggregate landing order tracks piece order. First
                        # groups split at 0.5MB so the PE starts ~11us in.
                        # wk/wv split in halves (kv0-15 gates only half A).
                        # Constants (rope/vt ~50us, masks ~62us) ride last.
                        def hs_pair(g, j):
                            return (
                                tiles[g][:, j * 2 * 1024 : (j + 1) * 2 * 1024],
                                hs2[:, g + 2 * j : g + 2 * j + 2,
                                    ca * TC : (cb + 1) * TC],
                            )

                        hw2 = HT * HD // 2
                        pieces = [
                            (wq_sbs[0][:, 0 : qw // 4], wq2[:, 0 : qw // 4]),
                            (tiles[0][:, 0:1024],
                             hs2[:, 0:1, ca * TC : (cb + 1) * TC]),
                            (tiles[0][:, 1024:2048],
                             hs2[:, 1:2, ca * TC : (cb + 1) * TC]),
                            (wq_sbs[0][:, qw // 4 : qw // 2],
                             wq2[:, qw // 4 : qw // 2]),
                            (tiles[0][:, 2048 : 2 * 2048],
                             hs2[:, 2:4, ca * TC : (cb + 1) * TC]),
                            (wq_sbs[0][:, qw // 2 : qw], wq2[:, qw // 2 : qw]),
                            hs_pair(0, 2),
                            hs_pair(0, 3),
                            wqp(1),
                            hsp_half(8, 0),
                            hsp_half(8, 1),
                            wqp(2),
                            hsp_half(16, 0),
                            hsp_half(16, 1),
                            (wk_sb[:, 0:hw2], wk2[:, 0:hw2]),
                            (wv_sb[:, 0:hw2], wv2[:, 0:hw2]),
                            wqp(3),
                            hsp_half(24, 0),
                            hsp_half(24, 1),
                            (wk_sb[:, hw2 : 2 * hw2], wk2[:, hw2 : 2 * hw2]),
                            (wv_sb[:, hw2 : 2 * hw2], wv2[:, hw2 : 2 * hw2]),
                            (cos_sb[:], cos_dram[:]),
                            (sin_sb[:], sin_dram[:]),
                            (id_sb[:], id_dram[:]),
                            (mask_sb[:], mask_dram[:]),
                            (mask2_sb[:, 0:TC], mask_dram[:]),
                            (mask2_sb[:, TC : 2 * TC], mask_dram[:]),
                        ]
                        for i, (dst, src) in enumerate(pieces):
                            eng = nc.sync if i % 2 == 0 else nc.scalar
                            eng.dma_start(out=dst, in_=src)
                    else:
                        for g in range(0, HT, 8):
                            eng = nc.sync if (g // 8) % 2 == 0 else nc.scalar
                            eng.dma_start(
                                out=tiles[g][:],
                                in_=hs2[:, g : g + 8, ca * TC : (cb + 1) * TC],
                            )
                    for g in range(0, HT, 8):
                        t = tiles[g]
                        for j in range(8):
                            hs_t[(ca, g + j)] = t[:, j * 1024 : j * 1024 + TC]
                            hs_t[(cb, g + j)] = t[:, j * 1024 + TC : (j + 1) * 1024]
                    return hs_t

                def rope(acc, dst, c, nm):
                    """dst = acc*cos + shift128(acc*sin2)."""
                    u = workp.tile([P, TC], bf16, tag="ru", name=f"ru_{nm}")
                    w = workp.tile([P, TC], bf16, tag="rw", name=f"rw_{nm}")
                    sslc = sin_sb[:, c * TC : (c + 1) * TC]
                    nc.vector.tensor_mul(u[64:128, :], acc[0:64, :], sslc[0:64, :])
                    nc.vector.tensor_mul(u[0:64, :], acc[64:128, :], sslc[64:128, :])
                    nc.vector.tensor_mul(w[:], acc[:], cos_sb[:, c * TC : (c + 1) * TC])
                    nc.vector.tensor_add(dst[:], w[:], u[:])

                def proj_chunk(c, hs_t, weave=None, kv_first=False,
                               q01_stage=None):
                    # Q pass first in steady state (its RoPE then hides
                    # under the KV pass). With q01_stage (chunk 1), heads
                    # 0/1 were already projected during the chunk-0 front:
                    # rope them from the SBUF stage (ready immediately) and
                    # only run the h2/h3 accumulation here.
                    def q_section():
                        if q01_stage is not None:
                            for o in range(2):
                                rope(q01_stage[o][:],
                                     qT_sb[:, o * S + c * TC : o * S + (c + 1) * TC],
                                     c, f"q_{c}_{o}")
                            aq23 = p2("p23", f"aq23_{c}")
                            qacc = {2: aq23[:, 0:TC], 3: aq23[:, TC : 2 * TC]}
                            # same split as the steady-state branch: h2's
                            # rope starts ~8.4us before h3's pass completes
                            for hp, o2 in enumerate((2, 3)):
                                for ht in range(HT):
                                    wqs = wq_sbs[ht // 8]
                                    nc.tensor.matmul(
                                        qacc[o2],
                                        wqs[:, (ht % 8) * 512 + o2 * P : (ht % 8) * 512 + (o2 + 1) * P],
                                        hs_t[(c, ht)],
                                        start=(ht == 0), stop=(ht == HT - 1),
                                    )
                                    if hp == 0 and ht == 3 and weave is not None:
                                        weave()
                                rope(qacc[o2],
                                     qT_sb[:, o2 * S + c * TC : o2 * S + (c + 1) * TC],
                                     c, f"q_{c}_{o2}")
                            return
                        else:
                            # two 2-head passes with rope emitted between:
                            # h0/h1's accumulation stops ~8.4us earlier, so
                            # their ropes start mid-section and the vector
                            # queue (woven tails + 4 ropes + rope-k, ~14.6us)
                            # drains well before attention needs qT/kT
                            aq01 = p2("p01", f"aq01_{c}")
                            aq23 = p2("p23", f"aq23_{c}")
                            qacc = {0: aq01[:, 0:TC], 1: aq01[:, TC : 2 * TC],
                                    2: aq23[:, 0:TC], 3: aq23[:, TC : 2 * TC]}
                            for hp, heads2 in enumerate(((0, 1), (2, 3))):
                                for ht in range(HT):
                                    wqs = wq_sbs[ht // 8]
                                    for o in heads2:
                                        nc.tensor.matmul(
                                            qacc[o],
                                            wqs[:, (ht % 8) * 512 + o * P : (ht % 8) * 512 + (o + 1) * P],
                                            hs_t[(c, ht)],
                                            start=(ht == 0), stop=(ht == HT - 1),
                                        )
                                    if hp == 0 and ht == 3 and weave is not None:
                                        weave()
                                for o in heads2:
                                    rope(qacc[o],
                                         qT_sb[:, o * S + c * TC : o * S + (c + 1) * TC],
                                         c, f"q_{c}_{o}")

                    def kv_section():
                        kacc = p1("pa", f"kacc_{c}")
                        vacc = p1("pb", f"vacc_{c}")
                        for ht in range(HT):
                            nc.tensor.matmul(
                                kacc[:], wk_sb[:, ht * P : (ht + 1) * P], hs_t[(c, ht)],
                                start=(ht == 0), stop=(ht == HT - 1),
                            )
                            nc.tensor.matmul(
                                vacc[:], wv_sb[:, ht * P : (ht + 1) * P], hs_t[(c, ht)],
                                start=(ht == 0), stop=(ht == HT - 1),
                            )
                        rope(kacc[:], kT_sb[:, c * TC : (c + 1) * TC], c, f"k_{c}")
                        vtmp = workp.tile([P, TC], bf16, tag="vtmp", name=f"vtmp_{c}")
                        nc.scalar.copy(vtmp[:], vacc[:])
                        for j in range(4):
                            tp = pm.tile(
                                [P, P], bf16, tag=("pc", "pd")[j % 2], bufs=1,
                                padded_shape=[P, TC], name=f"vt_{c}_{j}",
                            )
                            nc.tensor.transpose(tp[:], vtmp[:, j * P : (j + 1) * P], id_sb[:])
                            # j=0/1 read out on scalar (idle here): the vector
                            # queue is ~13us deep in RoPE + woven tail work,
                            # and transposes j+2 WAR-stall on these reads
                            eng = nc.scalar if j < 2 else nc.vector
                            if j < 2:
                                eng.copy(
                                    vnat_sb[:, (c * 4 + j) * P : (c * 4 + j + 1) * P],
                                    tp[:],
                                )
                            else:
                                eng.tensor_copy(
                                    vnat_sb[:, (c * 4 + j) * P : (c * 4 + j + 1) * P],
                                    tp[:],
                                )

                    if kv_first:
                        kv_section()
                        q_section()
                    else:
                        q_section()
                        kv_section()

                def proj_chunk0(hs_t):
                    """Chunk 0, DMA-bound front: interleave kv 8-ht groups
                    AND chunk 1's q heads 0/1 (same supertiles + weights,
                    zero extra DMA; accumulated in the pc/pd banks which are
                    free until the V transposes) into the Q pass where its
                    loads haven't landed yet. kv16-31 stays at the end so
                    RoPE-q (vector, ~9us) hides under it before attn0. The
                    q1 h0/h1 accumulators are staged to SBUF by the scalar
                    engine at the end (frees pc/pd for the transposes); their
                    RoPE runs from the stage early in proj1."""
                    c = 0
                    aq01 = p2("p01", "aq01_0")
                    aq23 = p2("p23", "aq23_0")
                    qacc = [aq01[:, 0:TC], aq01[:, TC : 2 * TC],
                            aq23[:, 0:TC], aq23[:, TC : 2 * TC]]
                    kacc = p1("pa", "kacc_0")
                    vacc = p1("pb", "vacc_0")
                    q1acc = [p1("pc", "q1acc_0"), p1("pd", "q1acc_1")]

                    def q_grp(gs):
                        for ht in range(gs, gs + 4):
                            wqs = wq_sbs[ht // 8]
                            for o in range(4):
                                nc.tensor.matmul(
                                    qacc[o],
                                    wqs[:, (ht % 8) * 512 + o * P : (ht % 8) * 512 + (o + 1) * P],
                                    hs_t[(c, ht)],
                                    start=(ht == 0), stop=(ht == HT - 1),
                                )

                    def q1_grp(gs):
                        for ht in range(gs, gs + 8):
                            wqs = wq_sbs[ht // 8]
                            for o in range(2):
                                nc.tensor.matmul(
                                    q1acc[o][:],
                                    wqs[:, (ht % 8) * 512 + o * P : (ht % 8) * 512 + (o + 1) * P],
                                    hs_t[(1, ht)],
                                    start=(ht == 0), stop=(ht == HT - 1),
                                )

                    def kv_grp(gs, ge):
                        # k then v: the vacc matmuls sit ~2us later on the
                        # PE, so the wv DMA piece can land later than wk's
                        for ht in range(gs, ge):
                            nc.tensor.matmul(
                                kacc[:], wk_sb[:, ht * P : (ht + 1) * P],
                                hs_t[(c, ht)],
                                start=(ht == 0), stop=(ht == HT - 1),
                            )
                        for ht in range(gs, ge):
                            nc.tensor.matmul(
                                vacc[:], wv_sb[:, ht * P : (ht + 1) * P],
                                hs_t[(c, ht)],
                                start=(ht == 0), stop=(ht == HT - 1),
                            )

                    # kv(0-16) is pure filler (its weights can land late), so
                    # it runs after the q/q1 stream; ropes then hide under
                    # q1(24)+kv(8-32) — a 16.8us window for 11.8us of vector
                    q_grp(0); q_grp(4)
                    q1_grp(0)
                    q_grp(8); q_grp(12)
                    q1_grp(8)
                    q_grp(16); q_grp(20)
                    q1_grp(16)
                    kv_grp(0, 8)
                    q_grp(24); q_grp(28)
                    for o in range(4):
                        rope(qacc[o], qT_sb[:, o * S + c * TC : o * S + (c + 1) * TC],
                             c, f"q_0_{o}")
                    q1_grp(24)
                    kv_grp(8, 16)
                    kv_grp(16, 32)
                    rope(kacc[:], kT_sb[:, 0:TC], c, "k_0")
                    # stage chunk-1 h0/h1 q accumulators to SBUF (scalar), so
                    # pc/pd free for the V transposes without waiting on the
                    # deep vector queue; their rope runs from here in proj1
                    qs1 = [workp.tile([P, TC], bf16, tag=f"qs1{o}",
                                      name=f"qs1_{o}") for o in range(2)]
                    nc.scalar.copy(qs1[0][:], q1acc[0][:])
                    nc.scalar.copy(qs1[1][:], q1acc[1][:])
                    vtmp = workp.tile([P, TC], bf16, tag="vtmp", name="vtmp_0")
                    nc.scalar.copy(vtmp[:], vacc[:])
                    for j in range(4):
                        tp = pm.tile(
                            [P, P], bf16, tag=("pc", "pd")[j % 2], bufs=1,
                            padded_shape=[P, TC], name=f"vt_0_{j}",
                        )
                        nc.tensor.transpose(tp[:], vtmp[:, j * P : (j + 1) * P], id_sb[:])
                        nc.vector.tensor_copy(
                            vnat_sb[:, j * P : (j + 1) * P], tp[:]
                        )
                    return qs1

                hs0 = emit_loads(0, 0, 1)
                qs1_01 = proj_chunk0(hs0)
                hs1 = emit_loads(1, 2, 3)  # prefetch during attn0/attn1
                t0 = attn(0)
                proj_chunk(1, hs0, weave=t0, q01_stage=qs1_01)
                t1 = attn(1)
                proj_chunk(2, hs1, weave=t1)
                t2 = attn(2)
                proj_chunk(3, hs1, weave=t2)
                t3 = attn(3)
                t3()

            # ---- Output projection: per-chunk passes in AllGather order.
            # One [128, 2048] read (4KB rows) per rank covers 4 heads; Wo
            # resident via two big contiguous DMAs.
            with (
                tc.tile_pool(name="wo", bufs=1) as wop,
                tc.tile_pool(name="workC", bufs=2) as workc,
            ):
                wo_sb = wop.tile([P, HT * NQH * HD], bf16)
                half = HT * NQH * HD // 2
                nc.sync.dma_start(out=wo_sb[:, 0:half], in_=wo2[:, 0:half])
                nc.scalar.dma_start(
                    out=wo_sb[:, half : 2 * half], in_=wo2[:, half : 2 * half]
                )

                for c in range(NT):
                    # double-buffer the accumulators across chunks (even:
                    # p01/p23, odd: pa..pd) so chunk c+1's first matmuls don't
                    # WAR-stall on chunk c's read-out copies
                    if c % 2 == 0:
                        y01 = p2("p01", f"y01_{c}")
                        y23 = p2("p23", f"y23_{c}")
                        ys = [y01[:, 0:TC], y01[:, TC : 2 * TC],
                              y23[:, 0:TC], y23[:, TC : 2 * TC]]
                    else:
                        ys = [p1(("pa", "pb", "pc", "pd")[o], f"y_{c}_{o}")[:]
                              for o in range(4)]
                    for r in range(N_CORES):
                        sup = workc.tile([P, NQH * TC], bf16, tag="ag",
                                         bufs=4, name=f"ag_{c}_{r}")
                        eng = nc.sync if r % 2 == 0 else nc.scalar
                        eng.dma_start(
                            out=sup[:], in_=ag_out[c][r * P : (r + 1) * P, :]
                        )
                        if r < N_CORES - 1:
                            for h in range(NQH):
                                ot = r * NQH + h
                                for o in range(4):
                                    nc.tensor.matmul(
                                        ys[o],
                                        wo_sb[:, ot * 512 + o * P : ot * 512 + (o + 1) * P],
                                        sup[:, h * TC : (h + 1) * TC],
                                        start=(ot == 0), stop=False,
                                    )
                        else:
                            # last rank o-major: ys[o] finishes ~12 matmuls
                            # before ys[o+1], so read-out + store DMA overlap
                            # the remaining accumulation
                            for o in range(4):
                                for h in range(NQH):
                                    ot = r * NQH + h
                                    nc.tensor.matmul(
                                        ys[o],
                                        wo_sb[:, ot * 512 + o * P : ot * 512 + (o + 1) * P],
                                        sup[:, h * TC : (h + 1) * TC],
                                        start=False, stop=(h == NQH - 1),
                                    )
                                yo = workc.tile([P, TC], bf16, tag=f"yo{o}",
                                                bufs=2, name=f"yo_{c}_{o}")
                                if o % 2 == 1:
                                    nc.scalar.copy(yo[:], ys[o])
                                else:
                                    nc.vector.tensor_copy(yo[:], ys[o])
                                deng = (nc.sync, nc.scalar, nc.gpsimd,
                                        nc.sync)[o]
                                deng.dma_start(
                                    out=out_ext[o * P : (o + 1) * P,
                                                c * TC : (c + 1) * TC],
                                    in_=yo[:],
                                )

    nc.finalize()
    return nc


def _get_built():
    global _BUILT
    if _BUILT is None:
        _BUILT = _build()
    return _BUILT


def _pack_pm(mT):
    """[H, W] -> [128, (H/128)*W]: row p holds the concatenation over h-tiles
    of mT[ht*128+p, :], so every SBUF-destined DMA row is wide+contiguous."""
    h, w = mT.shape
    return np.ascontiguousarray(
        mT.reshape(h // P, P, w).transpose(1, 0, 2).reshape(P, (h // P) * w)
    )


def make_in_maps(hidden_states, Wq, Wk, Wv, Wo):
    bf = ml_dtypes.bfloat16
    hs = np.asarray(hidden_states, dtype=np.float32).reshape(S, H)
    hs2 = _pack_pm(np.ascontiguousarray(hs.T).astype(bf))
    in_maps = []
    for c in range(N_CORES):
        in_maps.append(
            {
                "hs2": hs2,
                "wq2": _pack_pm(np.ascontiguousarray(np.asarray(Wq)[c * 512 : (c + 1) * 512].T).astype(bf)),
                "wk2": _pack_pm(np.ascontiguousarray(np.asarray(Wk)[c * 128 : (c + 1) * 128].T).astype(bf)),
                "wv2": _pack_pm(np.ascontiguousarray(np.asarray(Wv)[c * 128 : (c + 1) * 128].T).astype(bf)),
                "wo2": _pack_pm(np.ascontiguousarray(np.asarray(Wo)[c * 512 : (c + 1) * 512].T).astype(bf)),
            }
        )
    return in_maps


def kernel(hidden_states, Wq, Wk, Wv, Wo):
    from concourse.bass_utils import run_bass_kernel_spmd

    nc = _get_built()
    in_maps = make_in_maps(hidden_states, Wq, Wk, Wv, Wo)
    r = run_bass_kernel_spmd(nc, in_maps, list(range(N_CORES)))
    yT = np.concatenate([r.results[c]["out"] for c in range(N_CORES)], axis=0)
    return np.ascontiguousarray(yT.T).reshape(1, S, H).astype(np.float32)



# revision 73
# speedup vs baseline: 1.0243x; 1.0243x over previous
"""Mistral attention (B=1, S=2048, H=4096, 32 q-heads / 8 kv-heads GQA,
RoPE, causal) on 8 trn2 NeuronCores.

Sharding: tensor-parallel by kv head. Core c owns kv head c, q heads
4c..4c+3, and Wo rows 512c..512c+512. Attention outputs are AllGathered
per 512-token chunk; each core then computes its 512-row slice of the
output projection.

Schedule: chunk-serial and fully pipelined: for each 512-token chunk,
Q-projection -> (RoPE-q hidden under the K/V projection) -> attention
-> AllGather, so the four AllGathers cascade behind later chunks'
compute; per-chunk output-projection passes run at the end, each gated
only by its (long-finished) AllGather. The front is HBM-bound (~0.4
GB/us): chunk 0 interleaves kv 8-ht matmul groups AND chunk 1's q
heads 0/1 (same supertiles + weights, zero extra DMA, accumulated in
the otherwise-idle pc/pd banks and staged to SBUF) into the Q pass,
and the input DMA cascade is issued in exact PE-consumption order at
~0.5-1MB granularity over both hwdge queues, so the PE starts ~11us in
and streams behind the fill. proj1 then runs h2/h3 only, which also
frees it from the p01-bank WAR on attn0's last softmax exps. The AllGather payload is rank-major so
each o-proj read is one 4KB-row DMA per rank.

Attention is kt-outer/head-inner (ascending kt: diagonal tiles last,
hiding RoPE-k), with AV matmuls lagging scores by one kt step.
Masked diagonal tiles are trimmed to live query columns (N=512-128m).
Softmax denominators accumulate on the vector engine; the per-head
tail is entirely off the PE: GPSIMD partition_all_reduce -> DVE
reciprocal -> DVE multiply reading the AV psum bank directly, half
emitted inline and half woven into the next chunk's Q pass. Softmax
skips max-subtraction (unit-scale inputs). O-proj double-buffers its
accumulators across chunks (p01/p23 vs pa-pd), runs the last rank
o-major so read-out overlaps accumulation, and spreads store DMAs over
sync/scalar/gpsimd. Value path bf16; PSUM accumulation fp32; output
written bf16 and upcast on host.
"""

import math

import ml_dtypes
import numpy as np

P = 128
S = 2048
H = 4096
HD = 128
NQH = 4  # q heads per core
TC = 512  # token chunk
NT = S // TC  # 4 chunks
HT = H // P  # 32 h tiles
N_CORES = 8
ROPE_THETA = 10000.0

_BUILT = None


def _rope_tables():
    """cosT/sin2T in [hd partition, token free] layout.

    sin2T is the sin table pre-shifted/signed so that
    q_rot = q*cosT + shift128(q*sin2T), where shift128 swaps the two
    64-partition halves.
    """
    inv_freq = 1.0 / (ROPE_THETA ** (np.arange(0, HD, 2, dtype=np.float64) / HD))
    t = np.arange(S, dtype=np.float64)
    freqs = np.outer(t, inv_freq)  # [S, 64]
    emb = np.concatenate([freqs, freqs], axis=1)  # [S, HD]
    cosT = np.cos(emb).T.astype(np.float32)  # [HD, S]
    sinT = np.sin(emb).T.astype(np.float32)
    sin2T = sinT.copy()
    sin2T[64:] = -sin2T[64:]
    return (
        np.ascontiguousarray(cosT).astype(ml_dtypes.bfloat16),
        np.ascontiguousarray(sin2T).astype(ml_dtypes.bfloat16),
    )


def _mask():
    """[128, 512] bf16: mask[i, j] = (j >= i). Diagonal tile m of a chunk
    uses mask[:, 0:512-128m] against query columns [128m, 512)."""
    i = np.arange(P)[:, None]
    j = np.arange(TC)[None, :]
    return np.ascontiguousarray((j >= i).astype(np.float32)).astype(ml_dtypes.bfloat16)


def _build():
    import concourse.bacc as bacc
    import concourse.mybir as mybir
    import concourse.tile as tile

    f32 = mybir.dt.float32
    f32r = mybir.dt.float32r
    bf16 = mybir.dt.bfloat16

    nc = bacc.Bacc(
        "TRN2", target_bir_lowering=False, debug=False, num_devices=N_CORES
    )

    # Host-side repacked layouts: partition-major [128, ...] with wide
    # contiguous rows so DMA descriptors are 2KB+ (1KB rows cap a DMA
    # queue at ~90GB/s; the kernel front is load-bound otherwise).
    hs2 = nc.declare_dram_parameter("hs2", [P, HT, S], bf16, isOutput=False)
    wq2 = nc.declare_dram_parameter("wq2", [P, HT * NQH * HD], bf16, isOutput=False)
    wk2 = nc.declare_dram_parameter("wk2", [P, HT * HD], bf16, isOutput=False)
    wv2 = nc.declare_dram_parameter("wv2", [P, HT * HD], bf16, isOutput=False)
    wo2 = nc.declare_dram_parameter("wo2", [P, HT * NQH * HD], bf16, isOutput=False)
    out_ext = nc.declare_dram_parameter("out", [NQH * HD, S], bf16, isOutput=True)

    cosT_np, sin2T_np = _rope_tables()
    cos_dram = nc.inline_tensor(cosT_np, name="cosT")
    sin_dram = nc.inline_tensor(sin2T_np, name="sin2T")
    mask_dram = nc.inline_tensor(_mask(), name="mask")
    id_dram = nc.inline_tensor(np.eye(P).astype(ml_dtypes.bfloat16), name="ident")

    # rank-major AllGather payload: [hd, (head, token)] per rank, so the
    # o-proj phase can read one rank's 4 heads as a single 4KB-row DMA
    ag_in = [nc.dram_tensor(f"ag_in{c}", [P, NQH * TC], bf16) for c in range(NT)]
    ag_out = [
        nc.dram_tensor(f"ag_out{c}", [N_CORES * P, NQH * TC], bf16, addr_space="Shared")
        for c in range(NT)
    ]

    Exp = mybir.ActivationFunctionType.Exp
    SCALE = 1.0 / math.sqrt(HD)

    with tile.TileContext(nc) as tc:
        with (
            tc.tile_pool(name="const", bufs=1) as constp,
            tc.tile_pool(name="qkvout", bufs=1) as qp,
            tc.tile_pool(name="pmain", bufs=1, space="PSUM") as pm,
        ):
            # constants
            cos_sb = constp.tile([P, S], bf16)
            sin_sb = constp.tile([P, S], bf16)
            mask_sb = constp.tile([P, TC], bf16)
            mask2_sb = constp.tile([P, 2 * TC], bf16)
            id_sb = constp.tile([P, P], bf16)

            # persistent qkv outputs (bf16: PE runs bf16 at full rate)
            qT_sb = qp.tile([P, NQH * S], bf16)  # [hd, (head, t)]
            kT_sb = qp.tile([P, S], bf16)
            vnat_sb = qp.tile([P, S], bf16)  # [t%128, (ttile, hd)]

            # PSUM: 8 banks as two 2-bank tiles (p01, p23) and four 1-bank
            # tiles (pa..pd). Explicit tags keep cross-phase deps per-bank.
            def p2(tag, name):
                return pm.tile([P, 2 * TC], f32, tag=tag, bufs=1, name=name)

            def p1(tag, name):
                return pm.tile([P, TC], f32, tag=tag, bufs=1, name=name)

            with (
                tc.tile_pool(name="wqkv", bufs=1) as wp,
                tc.tile_pool(name="hsp", bufs=4) as hsp,
                tc.tile_pool(name="workA", bufs=2) as workp,
            ):
                # wq as four quarter tiles: each 8-ht group of the Q pass
                # gates on its own 1.05MB DMA, not half the 4.2MB load
                wq_sbs = [
                    wp.tile([P, HT * NQH * HD // 4], bf16, name=f"wq_sb{i}")
                    for i in range(4)
                ]
                wk_sb = wp.tile([P, HT * HD], bf16)
                wv_sb = wp.tile([P, HT * HD], bf16)

                def attn(c):
                    """Attention for chunk c + its AllGather."""
                    nkt = 4 * (c + 1)
                    avt = ["pa", "pb", "pc", "pd"]
                    av = [p1(avt[h], f"av_{c}_{h}") for h in range(NQH)]
                    ds = [
                        workp.tile([P, TC], bf16, tag=f"ds{h}", bufs=1,
                                   name=f"ds_{c}_{h}")
                        for h in range(NQH)
                    ]
                    pend = None  # (exs, coff, ncols, kt) awaiting AV matmuls

                    def emit_av(p):
                        exv, coff, kt = p
                        for h in range(NQH):
                            nc.tensor.matmul(
                                av[h][:, coff:TC],
                                vnat_sb[:, kt * P : (kt + 1) * P],
                                exv[h],
                                start=(kt == 0),
                                stop=(kt == nkt - 1),
                            )

                    for kt in range(nkt):
                        m = kt - 4 * c
                        ncols = TC - 128 * m if m > 0 else TC
                        coff = TC - ncols
                        scp = p2("p01", f"scp_{c}_{kt}")
                        scq = p2("p23", f"scq_{c}_{kt}")
                        halves = [
                            scp[:, 0:TC], scp[:, TC : 2 * TC],
                            scq[:, 0:TC], scq[:, TC : 2 * TC],
                        ]
                        for h in range(NQH):
                            nc.tensor.matmul(
                                halves[h][:, coff:TC],
                                kT_sb[:, kt * P : (kt + 1) * P],
                                qT_sb[:, h * S + c * TC + coff : h * S + (c + 1) * TC],
                                start=True,
                                stop=True,
                            )
                        if pend is not None:
                            emit_av(pend)
                        ex01 = workp.tile([P, 2 * TC], bf16, tag="ex", bufs=4,
                                          name=f"ex01_{c}_{kt}")
                        ex23 = workp.tile([P, 2 * TC], bf16, tag="ex", bufs=4,
                                          name=f"ex23_{c}_{kt}")
                        if coff == 0:
                            nc.scalar.activation(ex01[:], scp[:], Exp, scale=SCALE)
                            nc.scalar.activation(ex23[:], scq[:], Exp, scale=SCALE)
                        else:
                            for ex, sc in ((ex01, scp), (ex23, scq)):
                                nc.scalar.activation(
                                    ex[:, 0:ncols], sc[:, coff:TC], Exp, scale=SCALE
                                )
                                nc.scalar.activation(
                                    ex[:, TC : TC + ncols], sc[:, TC + coff : 2 * TC],
                                    Exp, scale=SCALE,
                                )
                        exv = [ex01[:, 0:ncols], ex01[:, TC : TC + ncols],
                               ex23[:, 0:ncols], ex23[:, TC : TC + ncols]]
                        if m == 0:
                            nc.vector.tensor_mul(ex01[:], ex01[:], mask2_sb[:])
                            nc.vector.tensor_mul(ex23[:], ex23[:], mask2_sb[:])
                        elif m > 0:
                            for h in range(NQH):
                                nc.vector.tensor_mul(
                                    exv[h], exv[h], mask_sb[:, 0:ncols]
                                )
                        for h in range(NQH):
                            if kt == 0:
                                nc.vector.tensor_copy(ds[h][:], exv[h])
                            else:
                                nc.vector.tensor_add(
                                    ds[h][:, coff:TC], ds[h][:, coff:TC], exv[h]
                                )
                        pend = (exv, coff, kt)
                    emit_av(pend)

                    # per-head tail, all off the PE: denominator via GPSIMD
                    # partition_all_reduce (idle engine; ~3us/head, hidden in
                    # the next chunk's Q pass), then one DVE divide reading
                    # the AV psum bank directly. Heads 0-1 emit here; heads
                    # 2-3 + the AllGather return as a closure the caller
                    # weaves into the next phase's instruction stream.
                    import bass_rust

                    def tail_pair(pair):
                        dnbs = {}
                        for h in pair:
                            dnb = workp.tile([P, TC], f32, tag=f"dnb{h}",
                                             bufs=1, name=f"dnb_{c}_{h}")
                            nc.gpsimd.partition_all_reduce(
                                dnb[:], ds[h][:], channels=P,
                                reduce_op=bass_rust.ReduceOp.add,
                            )
                            dnbs[h] = dnb
                        for h in pair:
                            rcf = workp.tile([P, TC], f32, tag=f"rcf{h % 2}",
                                             bufs=2, name=f"rcf_{c}_{h}")
                            nc.vector.reciprocal_approx_fast(rcf[:], dnbs[h][:])
                            ao = workp.tile([P, TC], bf16, tag="ao", bufs=4,
                                            name=f"ao_{c}_{h}")
                            nc.vector.tensor_mul(ao[:], av[h][:], rcf[:])
                            nc.sync.dma_start(
                                out=ag_in[c][:, h * TC : (h + 1) * TC], in_=ao[:]
                            )

                    tail_pair((0, 1))

                    def finish():
                        tail_pair((2, 3))
                        nc.gpsimd.collective_compute(
                            "AllGather",
                            mybir.AluOpType.bypass,
                            ins=[ag_in[c][:]],
                            outs=[ag_out[c][:]],
                            replica_groups=[list(range(N_CORES))],
                        )

                    return finish

                def emit_loads(pi, ca, cb):
                    """hs for the pair: four [128, 8x1024] supertiles (2KB
                    rows; each dma_start sprays 16 SDMA engines).

                    First pair: the front is HBM-bandwidth-bound (~0.36
                    GB/us), so pieces are issued in exact PE-consumption
                    order of the Q pass (wq quarter + its two hs half-
                    supertiles, per 8-ht group), round-robined over the three
                    DMA-capable queues (sync/scalar/gpsimd) so the aggregate
                    landing order tracks issue order. wk/wv/constants follow
                    (KV pass starts ~35us in, RoPE/masks later still)."""
                    hs_t = {}
                    tiles = {}
                    for g in range(0, HT, 8):
                        tiles[g] = hsp.tile([P, 8 * 2 * TC], bf16, tag="hs",
                                            name=f"hs_{ca}_{g}")
                    if pi == 0:
                        qw = HT * NQH * HD // 4

                        def hsp_half(g, h):
                            return (
                                tiles[g][:, h * 4 * 1024 : (h + 1) * 4 * 1024],
                                hs2[:, g + 4 * h : g + 4 * h + 4,
                                    ca * TC : (cb + 1) * TC],
                            )

                        def wqp(i):
                            return (wq_sbs[i][:], wq2[:, i * qw : (i + 1) * qw])

                        # Two-queue cascade (~0.2 GB/us each): pieces strictly
                        # in PE-consumption order of proj_chunk0's interleaved
                        # [q grp | kv grp] schedule, alternating queues so the
                        # aggregate landing order tracks piece order. First
                        # groups split at 0.5MB so the PE starts ~11us in.
                        # wk/wv split in halves (kv0-15 gates only half A).
                        # Constants (rope/vt ~50us, masks ~62us) ride last.
                        def hs_pair(g, j):
                            return (
                                tiles[g][:, j * 2 * 1024 : (j + 1) * 2 * 1024],
                                hs2[:, g + 2 * j : g + 2 * j + 2,
                                    ca * TC : (cb + 1) * TC],
                            )

                        hw2 = HT * HD // 2
                        pieces = [
                            (wq_sbs[0][:, 0 : qw // 4], wq2[:, 0 : qw // 4]),
                            (tiles[0][:, 0:1024],
                             hs2[:, 0:1, ca * TC : (cb + 1) * TC]),
                            (tiles[0][:, 1024:2048],
                             hs2[:, 1:2, ca * TC : (cb + 1) * TC]),
                            (wq_sbs[0][:, qw // 4 : qw // 2],
                             wq2[:, qw // 4 : qw // 2]),
                            (tiles[0][:, 2048 : 2 * 2048],
                             hs2[:, 2:4, ca * TC : (cb + 1) * TC]),
                            (wq_sbs[0][:, qw // 2 : qw], wq2[:, qw // 2 : qw]),
                            hs_pair(0, 2),
                            hs_pair(0, 3),
                            wqp(1),
                            hsp_half(8, 0),
                            hsp_half(8, 1),
                            wqp(2),
                            hsp_half(16, 0),
                            hsp_half(16, 1),
                            (wk_sb[:, 0:hw2], wk2[:, 0:hw2]),
                            (wv_sb[:, 0:hw2], wv2[:, 0:hw2]),
                            wqp(3),
                            hsp_half(24, 0),
                            hsp_half(24, 1),
                            (wk_sb[:, hw2 : 2 * hw2], wk2[:, hw2 : 2 * hw2]),
                            (wv_sb[:, hw2 : 2 * hw2], wv2[:, hw2 : 2 * hw2]),
                            (cos_sb[:], cos_dram[:]),
                            (sin_sb[:], sin_dram[:]),
                            (id_sb[:], id_dram[:]),
                            (mask_sb[:], mask_dram[:]),
                            (mask2_sb[:, 0:TC], mask_dram[:]),
                            (mask2_sb[:, TC : 2 * TC], mask_dram[:]),
                        ]
                        for i, (dst, src) in enumerate(pieces):
                            eng = nc.sync if i % 2 == 0 else nc.scalar
                            eng.dma_start(out=dst, in_=src)
                    else:
                        for g in range(0, HT, 8):
                            eng = nc.sync if (g // 8) % 2 == 0 else nc.scalar
                            eng.dma_start(
                                out=tiles[g][:],
                                in_=hs2[:, g : g + 8, ca * TC : (cb + 1) * TC],
                            )
                    for g in range(0, HT, 8):
                        t = tiles[g]
                        for j in range(8):
                            hs_t[(ca, g + j)] = t[:, j * 1024 : j * 1024 + TC]
                            hs_t[(cb, g + j)] = t[:, j * 1024 + TC : (j + 1) * 1024]
                    return hs_t

                def rope(acc, dst, c, nm):
                    """dst = acc*cos + shift128(acc*sin2)."""
                    u = workp.tile([P, TC], bf16, tag="ru", name=f"ru_{nm}")
                    w = workp.tile([P, TC], bf16, tag="rw", name=f"rw_{nm}")
                    sslc = sin_sb[:, c * TC : (c + 1) * TC]
                    nc.vector.tensor_mul(u[64:128, :], acc[0:64, :], sslc[0:64, :])
                    nc.vector.tensor_mul(u[0:64, :], acc[64:128, :], sslc[64:128, :])
                    nc.vector.tensor_mul(w[:], acc[:], cos_sb[:, c * TC : (c + 1) * TC])
                    nc.vector.tensor_add(dst[:], w[:], u[:])

                def proj_chunk(c, hs_t, weave=None, kv_first=False,
                               q01_stage=None):
                    # Q pass first in steady state (its RoPE then hides
                    # under the KV pass). With q01_stage (chunk 1), heads
                    # 0/1 were already projected during the chunk-0 front:
                    # rope them from the SBUF stage (ready immediately) and
                    # only run the h2/h3 accumulation here.
                    def q_section():
                        if q01_stage is not None:
                            for o in range(2):
                                rope(q01_stage[o][:],
                                     qT_sb[:, o * S + c * TC : o * S + (c + 1) * TC],
                                     c, f"q_{c}_{o}")
                            heads = (2, 3)
                            aq23 = p2("p23", f"aq23_{c}")
                            qacc = {2: aq23[:, 0:TC], 3: aq23[:, TC : 2 * TC]}
                        else:
                            # two 2-head passes with rope emitted between:
                            # h0/h1's accumulation stops ~8.4us earlier, so
                            # their ropes start mid-section and the vector
                            # queue (woven tails + 4 ropes + rope-k, ~14.6us)
                            # drains well before attention needs qT/kT
                            aq01 = p2("p01", f"aq01_{c}")
                            aq23 = p2("p23", f"aq23_{c}")
                            qacc = {0: aq01[:, 0:TC], 1: aq01[:, TC : 2 * TC],
                                    2: aq23[:, 0:TC], 3: aq23[:, TC : 2 * TC]}
                            for hp, heads2 in enumerate(((0, 1), (2, 3))):
                                for ht in range(HT):
                                    wqs = wq_sbs[ht // 8]
                                    for o in heads2:
                                        nc.tensor.matmul(
                                            qacc[o],
                                            wqs[:, (ht % 8) * 512 + o * P : (ht % 8) * 512 + (o + 1) * P],
                                            hs_t[(c, ht)],
                                            start=(ht == 0), stop=(ht == HT - 1),
                                        )
                                    if hp == 0 and ht == 3 and weave is not None:
                                        weave()
                                for o in heads2:
                                    rope(qacc[o],
                                         qT_sb[:, o * S + c * TC : o * S + (c + 1) * TC],
                                         c, f"q_{c}_{o}")
                            return
                        for ht in range(HT):
                            wqs = wq_sbs[ht // 8]
                            for o in heads:
                                nc.tensor.matmul(
                                    qacc[o],
                                    wqs[:, (ht % 8) * 512 + o * P : (ht % 8) * 512 + (o + 1) * P],
                                    hs_t[(c, ht)],
                                    start=(ht == 0), stop=(ht == HT - 1),
                                )
                            if ht == 3 and weave is not None:
                                weave()
                        for o in heads:
                            rope(qacc[o],
                                 qT_sb[:, o * S + c * TC : o * S + (c + 1) * TC],
                                 c, f"q_{c}_{o}")

                    def kv_section():
                        kacc = p1("pa", f"kacc_{c}")
                        vacc = p1("pb", f"vacc_{c}")
                        for ht in range(HT):
                            nc.tensor.matmul(
                                kacc[:], wk_sb[:, ht * P : (ht + 1) * P], hs_t[(c, ht)],
                                start=(ht == 0), stop=(ht == HT - 1),
                            )
                            nc.tensor.matmul(
                                vacc[:], wv_sb[:, ht * P : (ht + 1) * P], hs_t[(c, ht)],
                                start=(ht == 0), stop=(ht == HT - 1),
                            )
                        rope(kacc[:], kT_sb[:, c * TC : (c + 1) * TC], c, f"k_{c}")
                        vtmp = workp.tile([P, TC], bf16, tag="vtmp", name=f"vtmp_{c}")
                        nc.scalar.copy(vtmp[:], vacc[:])
                        for j in range(4):
                            tp = pm.tile(
                                [P, P], bf16, tag=("pc", "pd")[j % 2], bufs=1,
                                padded_shape=[P, TC], name=f"vt_{c}_{j}",
                            )
                            nc.tensor.transpose(tp[:], vtmp[:, j * P : (j + 1) * P], id_sb[:])
                            # j=0/1 read out on scalar (idle here): the vector
                            # queue is ~13us deep in RoPE + woven tail work,
                            # and transposes j+2 WAR-stall on these reads
                            eng = nc.scalar if j < 2 else nc.vector
                            if j < 2:
                                eng.copy(
                                    vnat_sb[:, (c * 4 + j) * P : (c * 4 + j + 1) * P],
                                    tp[:],
                                )
                            else:
                                eng.tensor_copy(
                                    vnat_sb[:, (c * 4 + j) * P : (c * 4 + j + 1) * P],
                                    tp[:],
                                )

                    if kv_first:
                        kv_section()
                        q_section()
                    else:
                        q_section()
                        kv_section()

                def proj_chunk0(hs_t):
                    """Chunk 0, DMA-bound front: interleave kv 8-ht groups
                    AND chunk 1's q heads 0/1 (same supertiles + weights,
                    zero extra DMA; accumulated in the pc/pd banks which are
                    free until the V transposes) into the Q pass where its
                    loads haven't landed yet. kv16-31 stays at the end so
                    RoPE-q (vector, ~9us) hides under it before attn0. The
                    q1 h0/h1 accumulators are staged to SBUF by the scalar
                    engine at the end (frees pc/pd for the transposes); their
                    RoPE runs from the stage early in proj1."""
                    c = 0
                    aq01 = p2("p01", "aq01_0")
                    aq23 = p2("p23", "aq23_0")
                    qacc = [aq01[:, 0:TC], aq01[:, TC : 2 * TC],
                            aq23[:, 0:TC], aq23[:, TC : 2 * TC]]
                    kacc = p1("pa", "kacc_0")
                    vacc = p1("pb", "vacc_0")
                    q1acc = [p1("pc", "q1acc_0"), p1("pd", "q1acc_1")]

                    def q_grp(gs):
                        for ht in range(gs, gs + 4):
                            wqs = wq_sbs[ht // 8]
                            for o in range(4):
                                nc.tensor.matmul(
                                    qacc[o],
                                    wqs[:, (ht % 8) * 512 + o * P : (ht % 8) * 512 + (o + 1) * P],
                                    hs_t[(c, ht)],
                                    start=(ht == 0), stop=(ht == HT - 1),
                                )

                    def q1_grp(gs):
                        for ht in range(gs, gs + 8):
                            wqs = wq_sbs[ht // 8]
                            for o in range(2):
                                nc.tensor.matmul(
                                    q1acc[o][:],
                                    wqs[:, (ht % 8) * 512 + o * P : (ht % 8) * 512 + (o + 1) * P],
                                    hs_t[(1, ht)],
                                    start=(ht == 0), stop=(ht == HT - 1),
                                )

                    def kv_grp(gs, ge):
                        # k then v: the vacc matmuls sit ~2us later on the
                        # PE, so the wv DMA piece can land later than wk's
                        for ht in range(gs, ge):
                            nc.tensor.matmul(
                                kacc[:], wk_sb[:, ht * P : (ht + 1) * P],
                                hs_t[(c, ht)],
                                start=(ht == 0), stop=(ht == HT - 1),
                            )
                        for ht in range(gs, ge):
                            nc.tensor.matmul(
                                vacc[:], wv_sb[:, ht * P : (ht + 1) * P],
                                hs_t[(c, ht)],
                                start=(ht == 0), stop=(ht == HT - 1),
                            )

                    # kv(0-16) is pure filler (its weights can land late), so
                    # it runs after the q/q1 stream; ropes then hide under
                    # q1(24)+kv(8-32) — a 16.8us window for 11.8us of vector
                    q_grp(0); q_grp(4)
                    q1_grp(0)
                    q_grp(8); q_grp(12)
                    q1_grp(8)
                    q_grp(16); q_grp(20)
                    q1_grp(16)
                    kv_grp(0, 8)
                    q_grp(24); q_grp(28)
                    for o in range(4):
                        rope(qacc[o], qT_sb[:, o * S + c * TC : o * S + (c + 1) * TC],
                             c, f"q_0_{o}")
                    q1_grp(24)
                    kv_grp(8, 16)
                    kv_grp(16, 32)
                    rope(kacc[:], kT_sb[:, 0:TC], c, "k_0")
                    # stage chunk-1 h0/h1 q accumulators to SBUF (scalar), so
                    # pc/pd free for the V transposes without waiting on the
                    # deep vector queue; their rope runs from here in proj1
                    qs1 = [workp.tile([P, TC], bf16, tag=f"qs1{o}",
                                      name=f"qs1_{o}") for o in range(2)]
                    nc.scalar.copy(qs1[0][:], q1acc[0][:])
                    nc.scalar.copy(qs1[1][:], q1acc[1][:])
                    vtmp = workp.tile([P, TC], bf16, tag="vtmp", name="vtmp_0")
                    nc.scalar.copy(vtmp[:], vacc[:])
                    for j in range(4):
                        tp = pm.tile(
                            [P, P], bf16, tag=("pc", "pd")[j % 2], bufs=1,
                            padded_shape=[P, TC], name=f"vt_0_{j}",
                        )
                        nc.tensor.transpose(tp[:], vtmp[:, j * P : (j + 1) * P], id_sb[:])
                        nc.vector.tensor_copy(
                            vnat_sb[:, j * P : (j + 1) * P], tp[:]
                        )
                    return qs1

                hs0 = emit_loads(0, 0, 1)
                qs1_01 = proj_chunk0(hs0)
                hs1 = emit_loads(1, 2, 3)  # prefetch during attn0/attn1
                t0 = attn(0)
                proj_chunk(1, hs0, weave=t0, q01_stage=qs1_01)
                t1 = attn(1)
                proj_chunk(2, hs1, weave=t1)
                t2 = attn(2)
                proj_chunk(3, hs1, weave=t2)
                t3 = attn(3)
                t3()

            # ---- Output projection: per-chunk passes in AllGather order.
            # One [128, 2048] read (4KB rows) per rank covers 4 heads; Wo
            # resident via two big contiguous DMAs.
            with (
                tc.tile_pool(name="wo", bufs=1) as wop,
                tc.tile_pool(name="workC", bufs=2) as workc,
            ):
                wo_sb = wop.tile([P, HT * NQH * HD], bf16)
                half = HT * NQH * HD // 2
                nc.sync.dma_start(out=wo_sb[:, 0:half], in_=wo2[:, 0:half])
                nc.scalar.dma_start(
                    out=wo_sb[:, half : 2 * half], in_=wo2[:, half : 2 * half]
                )

                for c in range(NT):
                    # double-buffer the accumulators across chunks (even:
                    # p01/p23, odd: pa..pd) so chunk c+1's first matmuls don't
                    # WAR-stall on chunk c's read-out copies
                    if c % 2 == 0:
                        y01 = p2("p01", f"y01_{c}")
                        y23 = p2("p23", f"y23_{c}")
                        ys = [y01[:, 0:TC], y01[:, TC : 2 * TC],
                              y23[:, 0:TC], y23[:, TC : 2 * TC]]
                    else:
                        ys = [p1(("pa", "pb", "pc", "pd")[o], f"y_{c}_{o}")[:]
                              for o in range(4)]
                    for r in range(N_CORES):
                        sup = workc.tile([P, NQH * TC], bf16, tag="ag",
                                         bufs=4, name=f"ag_{c}_{r}")
                        eng = nc.sync if r % 2 == 0 else nc.scalar
                        eng.dma_start(
                            out=sup[:], in_=ag_out[c][r * P : (r + 1) * P, :]
                        )
                        if r < N_CORES - 1:
                            for h in range(NQH):
                                ot = r * NQH + h
                                for o in range(4):
                                    nc.tensor.matmul(
                                        ys[o],
                                        wo_sb[:, ot * 512 + o * P : ot * 512 + (o + 1) * P],
                                        sup[:, h * TC : (h + 1) * TC],
                                        start=(ot == 0), stop=False,
                                    )
                        else:
                            # last rank o-major: ys[o] finishes ~12 matmuls
                            # before ys[o+1], so read-out + store DMA overlap
                            # the remaining accumulation
                            for o in range(4):
                                for h in range(NQH):
                                    ot = r * NQH + h
                                    nc.tensor.matmul(
                                        ys[o],
                                        wo_sb[:, ot * 512 + o * P : ot * 512 + (o + 1) * P],
                                        sup[:, h * TC : (h + 1) * TC],
                                        start=False, stop=(h == NQH - 1),
                                    )
                                yo = workc.tile([P, TC], bf16, tag=f"yo{o}",
                                                bufs=2, name=f"yo_{c}_{o}")
                                if o % 2 == 1:
                                    nc.scalar.copy(yo[:], ys[o])
                                else:
                                    nc.vector.tensor_copy(yo[:], ys[o])
                                deng = (nc.sync, nc.scalar, nc.gpsimd,
                                        nc.sync)[o]
                                deng.dma_start(
                                    out=out_ext[o * P : (o + 1) * P,
                                                c * TC : (c + 1) * TC],
                                    in_=yo[:],
                                )

    nc.finalize()
    return nc


def _get_built():
    global _BUILT
    if _BUILT is None:
        _BUILT = _build()
    return _BUILT


def _pack_pm(mT):
    """[H, W] -> [128, (H/128)*W]: row p holds the concatenation over h-tiles
    of mT[ht*128+p, :], so every SBUF-destined DMA row is wide+contiguous."""
    h, w = mT.shape
    return np.ascontiguousarray(
        mT.reshape(h // P, P, w).transpose(1, 0, 2).reshape(P, (h // P) * w)
    )


def make_in_maps(hidden_states, Wq, Wk, Wv, Wo):
    bf = ml_dtypes.bfloat16
    hs = np.asarray(hidden_states, dtype=np.float32).reshape(S, H)
    hs2 = _pack_pm(np.ascontiguousarray(hs.T).astype(bf))
    in_maps = []
    for c in range(N_CORES):
        in_maps.append(
            {
                "hs2": hs2,
                "wq2": _pack_pm(np.ascontiguousarray(np.asarray(Wq)[c * 512 : (c + 1) * 512].T).astype(bf)),
                "wk2": _pack_pm(np.ascontiguousarray(np.asarray(Wk)[c * 128 : (c + 1) * 128].T).astype(bf)),
                "wv2": _pack_pm(np.ascontiguousarray(np.asarray(Wv)[c * 128 : (c + 1) * 128].T).astype(bf)),
                "wo2": _pack_pm(np.ascontiguousarray(np.asarray(Wo)[c * 512 : (c + 1) * 512].T).astype(bf)),
            }
        )
    return in_maps


def kernel(hidden_states, Wq, Wk, Wv, Wo):
    from concourse.bass_utils import run_bass_kernel_spmd

    nc = _get_built()
    in_maps = make_in_maps(hidden_states, Wq, Wk, Wv, Wo)
    r = run_bass_kernel_spmd(nc, in_maps, list(range(N_CORES)))
    yT = np.concatenate([r.results[c]["out"] for c in range(N_CORES)], axis=0)
    return np.ascontiguousarray(yT.T).reshape(1, S, H).astype(np.float32)

